# revision 9
# baseline (speedup 1.0000x reference)
"""GNN edge attention-score kernel for 8 TRN2 NeuronCores.

reference:  Q = x @ W.T ;  scores[e] = dot(Q[src[e]], Q[dst[e]])
  x [50000, 128] f32, W [128, 128] f32, edge_index [2, 1600000] i64
  out: scores [1600000] f32

Strategy (edge-parallel, x/W replicated):
  - host: transpose x -> xT [128, Npad] (feature-major), W -> W.T,
    partition edges globally into 4 (src%2, dst%2) parity groups, deal each
    group evenly across the 8 cores, pad per-core groups to a fixed tile
    count so all cores run an identical (SPMD) instruction stream.
  - device (per core): Q = x @ W.T via PE (xT tile = stationary, W.T
    moving), staged through SBUF to an HBM scratch buffer in node-major
    rows (512 B).  Then per edge tile: SWDGE dma_gather of Q[src] and
    Q[dst] rows (row i of a tile lands on partition i%128), DVE multiply
    + free-axis reduce -> per-edge scores.  The int16 gather-index limit
    (32767) is dodged by gathering with a 1024 B stride (node pairs) and
    choosing the even/odd 512 B half per parity group.
  - host: inverse-permute the padded per-core scores back to edge order.
"""

import sys

sys.path.insert(0, "/opt/trn_rl_repo")

import numpy as np

N_CORES = 8
N_FEAT = 128
N_NODES = 50000
N_EDGES = 1600000

TILE_E = 2048  # edges per gather tile (mult of 128)
SINGLE_PACKET = False
EB = TILE_E // 128  # edge blocks per tile
TILES_PER_GROUP = 25  # per parity group per core
NQ = (N_NODES + 127) // 128  # 391 Q tiles
N_PAD = NQ * 128  # 50048
N_PAIRS = N_PAD // 2


def build_graph(nq, tiles_per_group, tile_e):
    import concourse.bacc as bacc
    import concourse.mybir as mybir
    from concourse.library_config import mlp

    f32 = mybir.dt.float32
    i16 = mybir.dt.int16

    n_pad = nq * 128
    nt = 4 * tiles_per_group
    eb = tile_e // 128
    idx_cols = nt * tile_e // 16

    nc = bacc.Bacc("TRN2")
    xt_ext = nc.declare_dram_parameter("xt", [128, n_pad], f32, isOutput=False)
    wt_ext = nc.declare_dram_parameter("wt", [128, 128], f32, isOutput=False)
    si_ext = nc.declare_dram_parameter("si", [128, idx_cols], i16, isOutput=False)
    di_ext = nc.declare_dram_parameter("di", [128, idx_cols], i16, isOutput=False)
    out_ext = nc.declare_dram_parameter("out", [128, nt * eb], f32, isOutput=True)
    q_hbm = nc.dram_tensor("qbuf", [nq, 128, 128], f32)

    # strided pair views of Q rows: row-pair p covers nodes 2p / 2p+1
    q_pairs = q_hbm[:].rearrange("t p c -> (t p c)").rearrange("(n x) -> n x", x=256)
    q_views = [q_pairs[:, 0:128], q_pairs[:, 128:256]]  # even / odd nodes

    from contextlib import ExitStack

    with ExitStack() as ctx:
        wt_sb = ctx.enter_context(nc.sbuf_tensor("wt_sb", [128, 128], f32))
        si_sb = ctx.enter_context(nc.sbuf_tensor("si_sb", [128, idx_cols], i16))
        di_sb = ctx.enter_context(nc.sbuf_tensor("di_sb", [128, idx_cols], i16))
        xtb = ctx.enter_context(nc.sbuf_tensor("xtb", [128, 2, 128], f32))
        q_sb = ctx.enter_context(nc.sbuf_tensor("q_sb", [128, 2, 128], f32))
        S = ctx.enter_context(nc.sbuf_tensor("S", [128, 2, eb, 128], f32))
        D = ctx.enter_context(nc.sbuf_tensor("D", [128, 2, eb, 128], f32))
        P = ctx.enter_context(nc.sbuf_tensor("P", [128, 3, eb, 128], f32))
        out_sc = ctx.enter_context(nc.sbuf_tensor("out_sc", [128, nt * eb], f32))
        qp0 = ctx.enter_context(nc.psum_tensor("qp0", [128, 512], f32))
        qp1 = ctx.enter_context(nc.psum_tensor("qp1", [128, 512], f32))
        ld = ctx.enter_context(nc.semaphore("ld"))  # si/di/wt loads
        xl0 = ctx.enter_context(nc.semaphore("xl0"))  # xt loads even
        xl1 = ctx.enter_context(nc.semaphore("xl1"))
        pe = ctx.enter_context(nc.semaphore("pe"))  # matmuls
        cpA = ctx.enter_context(nc.semaphore("cpA"))  # ACT copies (even)
        cpD = ctx.enter_context(nc.semaphore("cpD"))  # DVE copies (odd)
        qs0 = ctx.enter_context(nc.semaphore("qs0"))  # q stores even
        qs1 = ctx.enter_context(nc.semaphore("qs1"))
        gt0 = ctx.enter_context(nc.semaphore("gt0"))  # gathers even tiles
        gt1 = ctx.enter_context(nc.semaphore("gt1"))
        ed = ctx.enter_context(nc.semaphore("ed"))  # DVE edge ops
        os_ = ctx.enter_context(nc.semaphore("os"))
        block = ctx.enter_context(nc.Block())
        qp = [qp0, qp1]
        xl = [xl0, xl1]
        qs = [qs0, qs1]
        gt = [gt0, gt1]

        @block.sync
        def _(sync):
            sync.dma_start(out=wt_sb[:], in_=wt_ext[:]).then_inc(ld, 16)
            sync.dma_start(out=si_sb[:], in_=si_ext[:]).then_inc(ld, 16)
            sync.dma_start(out=di_sb[:], in_=di_ext[:]).then_inc(ld, 16)
            for k in range(nq):
                if k >= 2:
                    sync.wait_ge(pe, k - 1)  # matmul k-2 done -> xtb[k%2] free
                sync.dma_start(
                    out=xtb[:, k % 2, :], in_=xt_ext[:, k * 128 : (k + 1) * 128]
                ).then_inc(xl[k % 2], 16)
                if k >= 1:
                    j = k - 1
                    if j % 2 == 0:
                        sync.wait_ge(cpA, j // 2 + 1)
                    else:
                        sync.wait_ge(cpD, j // 2 + 1)
                    sync.dma_start(out=q_hbm[j], in_=q_sb[:, j % 2, :]).then_inc(
                        qs[j % 2], 16
                    )
            j = nq - 1
            if j % 2 == 0:
                sync.wait_ge(cpA, j // 2 + 1)
            else:
                sync.wait_ge(cpD, j // 2 + 1)
            sync.dma_start(out=q_hbm[j], in_=q_sb[:, j % 2, :]).then_inc(
                qs[j % 2], 16
            )
            # final scores out
            sync.wait_ge(ed, 2 * nt)
            sync.dma_start(out=out_ext[:], in_=out_sc[:]).then_inc(os_, 16)
            sync.wait_ge(os_, 16)

        @block.tensor
        def _(tensor):
            tensor.wait_ge(ld, 48)  # wt loaded (also si/di; single sem)
            for k in range(nq):
                tensor.wait_ge(xl[k % 2], 16 * (k // 2 + 1))
                if k >= 2:
                    kk = k - 2
                    if kk % 2 == 0:
                        tensor.wait_ge(cpA, kk // 2 + 1)  # psum bank evacuated
                    else:
                        tensor.wait_ge(cpD, kk // 2 + 1)
                tensor.matmul(
                    qp[k % 2][:, 0:128], lhsT=xtb[:, k % 2, :], rhs=wt_sb[:]
                ).then_inc(pe, 1)

        @block.scalar
        def _(scalar):
            for j in range(0, nq, 2):
                scalar.wait_ge(pe, j + 1)
                if j >= 2:
                    scalar.wait_ge(qs[0], 16 * ((j - 2) // 2 + 1))  # q_sb[0] stored
                scalar.copy(out=q_sb[:, 0, :], in_=qp[0][:, 0:128]).then_inc(cpA, 1)

        @block.vector
        def _(vector):
            for j in range(1, nq, 2):
                vector.wait_ge(pe, j + 1)
                if j >= 2:
                    vector.wait_ge(qs[1], 16 * ((j - 2) // 2 + 1))
                vector.tensor_copy(out=q_sb[:, 1, :], in_=qp[1][:, 0:128]).then_inc(
                    cpD, 1
                )
            import concourse.mybir as mybir

            def red(t, mult_count):
                # DVE is pipelined: a same-engine RAW on P needs an explicit
                # (instantly-satisfied) wait on the producing mult
                vector.wait_ge(ed, mult_count)
                vector.tensor_reduce(
                    out=out_sc[:, t * eb : (t + 1) * eb],
                    in_=P[:, t % 3, :, :],
                    axis=mybir.AxisListType.X,
                    op=mybir.AluOpType.add,
                ).then_inc(ed, 1)

            # software pipeline: mult(t) then red(t-1) so same-buffer DVE ops
            # are never pipeline-adjacent; mult count m_t = 2t (m_0 = 1),
            # red(t-1) count = 2t+1
            for t in range(nt):
                vector.wait_ge(gt[t % 2], 32 * (t // 2 + 1))
                if t >= 3:
                    # WAR: P[t%3] was read by red(t-3), op count 2(t-2)+1
                    vector.wait_ge(ed, 2 * (t - 2) + 1)
                vector.tensor_tensor(
                    out=P[:, t % 3, :, :],
                    in0=S[:, t % 2, :, :],
                    in1=D[:, t % 2, :, :],
                    op=mybir.AluOpType.mult,
                ).then_inc(ed, 1)
                if t >= 1:
                    red(t - 1, max(1, 2 * (t - 1)))
            red(nt - 1, 2 * (nt - 1))

        @block.gpsimd
        def _(gpsimd):
            gpsimd.load_library(mlp)
            gpsimd.wait_ge(ld, 48)  # si/di in SBUF
            gpsimd.wait_ge(qs[0], 16 * ((nq + 1) // 2))  # all Q rows written
            gpsimd.wait_ge(qs[1], 16 * (nq // 2))
            ic = tile_e // 16  # idx cols per tile
            for t in range(nt):
                if t >= 2:
                    # mult(t-2) (which read S/D[t%2]) done: it is op 2(t-2)
                    # in the DVE edge sequence (op 1 for t-2 == 0)
                    gpsimd.wait_ge(ed, max(1, 2 * (t - 2)))
                g = t // tiles_per_group
                sp, dp = g >> 1, g & 1
                gpsimd.dma_gather(
                    S[:, t % 2, :, :],
                    q_views[sp],
                    si_sb[:, t * ic : (t + 1) * ic],
                    tile_e,
                    tile_e,
                    128,
                    elem_step=256,
                    single_packet=SINGLE_PACKET,
                ).then_inc(gt[t % 2], 16)
                gpsimd.dma_gather(
                    D[:, t % 2, :, :],
                    q_views[dp],
                    di_sb[:, t * ic : (t + 1) * ic],
                    tile_e,
                    tile_e,
                    128,
                    elem_step=256,
                    single_packet=SINGLE_PACKET,
                ).then_inc(gt[t % 2], 16)

    nc.compile()
    return nc


_GRAPH_CACHE = {}


def _get_graph(nq, tiles_per_group, tile_e):
    key = (nq, tiles_per_group, tile_e)
    if key not in _GRAPH_CACHE:
        _GRAPH_CACHE[key] = build_graph(nq, tiles_per_group, tile_e)
    return _GRAPH_CACHE[key]


def _wrap_idx(arr):
    """[n] int16 -> SWDGE SBUF index layout [128, n//16]."""
    a = arr.reshape(-1, 16).T  # [16, n/16]
    return np.tile(a, (8, 1)).copy()  # replicate for the 8 Q7 cores


def shard_edges(edge_index, tiles_per_group=TILES_PER_GROUP, tile_e=TILE_E):
    """Returns per-core (si, di) index planes + per-core edge positions."""
    cap = tiles_per_group * tile_e
    s = edge_index[0].astype(np.int64)
    d = edge_index[1].astype(np.int64)
    g = ((s & 1) << 1) | (d & 1)
    si_planes, di_planes, positions = (
        [[] for _ in range(N_CORES)],
        [[] for _ in range(N_CORES)],
        [[] for _ in range(N_CORES)],
    )
    for grp in range(4):
        (pos,) = np.nonzero(g == grp)
        chunks = np.array_split(pos, N_CORES)
        for c, ch in enumerate(chunks):
            n = len(ch)
            assert n <= cap, f"group {grp} core {c}: {n} > cap {cap}"
            sh = np.zeros(cap, np.int16)
            dh = np.zeros(cap, np.int16)
            sh[:n] = (s[ch] >> 1).astype(np.int16)
            dh[:n] = (d[ch] >> 1).astype(np.int16)
            pp = np.full(cap, -1, np.int64)
            pp[:n] = ch
            si_planes[c].append(sh)
            di_planes[c].append(dh)
            positions[c].append(pp)
    out = []
    for c in range(N_CORES):
        out.append(
            (
                _wrap_idx(np.concatenate(si_planes[c])),
                _wrap_idx(np.concatenate(di_planes[c])),
                np.concatenate(positions[c]),
            )
        )
    return out


def kernel(x, edge_index, W):
    from concourse.bass_utils import run_bass_kernel_spmd

    xt = np.zeros((N_FEAT, N_PAD), np.float32)
    xt[:, :N_NODES] = np.ascontiguousarray(x.astype(np.float32).T)
    wt = np.ascontiguousarray(W.astype(np.float32).T)

    shards = shard_edges(edge_index)
    in_maps = [
        {"xt": xt, "wt": wt, "si": si, "di": di} for (si, di, _pos) in shards
    ]

    nc = _get_graph(NQ, TILES_PER_GROUP, TILE_E)
    res = run_bass_kernel_spmd(nc, in_maps, core_ids=list(range(N_CORES)))
    global LAST_RESULT
    LAST_RESULT = res

    nt = 4 * TILES_PER_GROUP
    scores = np.empty(N_EDGES, np.float32)
    for c in range(N_CORES):
        buf = res.results[c]["out"]  # [128, nt*EB]
        flat = buf.reshape(128, nt, EB).transpose(1, 2, 0).reshape(-1)
        pos = shards[c][2]
        m = pos >= 0
        scores[pos[m]] = flat[m]
    return scores


# revision 21
# speedup vs baseline: 3.2687x; 3.2687x over previous
"""GNN edge attention-score kernel for 8 TRN2 NeuronCores.

reference:  Q = x @ W.T ;  scores[e] = dot(Q[src[e]], Q[dst[e]])
  x [50000, 128] f32, W [128, 128] f32, edge_index [2, 1600000] i64
  out: scores [1600000] f32

Strategy (edge-parallel, x/W replicated):
  - host: transpose x -> xT [128, Npad] (feature-major), W -> W.T,
    partition edges globally into 4 (src%2, dst%2) parity groups, deal each
    group evenly across the 8 cores, pad per-core groups to a fixed tile
    count so all cores run an identical (SPMD) instruction stream.
  - device (per core): Q = x @ W.T via PE (xT tile = stationary, W.T
    moving), staged through SBUF to an HBM scratch buffer in node-major
    rows (512 B).  Then per edge tile: SWDGE dma_gather of Q[src] and
    Q[dst] rows (row i of a tile lands on partition i%128), DVE multiply
    + free-axis reduce -> per-edge scores.  The int16 gather-index limit
    (32767) is dodged by gathering with a 1024 B stride (node pairs) and
    choosing the even/odd 512 B half per parity group.
  - host: inverse-permute the padded per-core scores back to edge order.

The builder supports `reps` (repeat the whole pipeline R times inside one
NEFF) purely for timing: device exec time = slope of T(R).
"""

import sys

sys.path.insert(0, "/opt/trn_rl_repo")

import numpy as np

N_CORES = 8
N_FEAT = 128
N_NODES = 50000
N_EDGES = 1600000

TILE_E = 2048  # edges per gather tile (mult of 128)
SINGLE_PACKET = False
EB = TILE_E // 128  # edge blocks per tile
TILES_PER_GROUP = 25  # per parity group per core
NQ = (N_NODES + 127) // 128  # 391 Q tiles
N_PAD = NQ * 128  # 50048
N_PAIRS = N_PAD // 2


def build_graph(nq, tiles_per_group, tile_e, do_q=True, do_edges=True, reps=1, do_dve=True, n_gath=2, elem_words=128, gather_mode="swdge", q_bf16=False, overlap_q=False):
    import concourse.bacc as bacc
    import concourse.mybir as mybir
    from concourse.library_config import mlp

    f32 = mybir.dt.float32
    i16 = mybir.dt.int16

    n_pad = nq * 128
    nt = 4 * tiles_per_group
    eb = tile_e // 128
    idx_cols = nt * tile_e // 16
    NQT = reps * nq if do_q else 0  # total Q tiles
    NTT = reps * nt if do_edges else 0  # total edge tiles

    def evens(n):  # number of even integers in [0, n)
        return (n + 1) // 2

    def odds(n):
        return n // 2

    bf16 = mybir.dt.bfloat16
    xdt = bf16 if q_bf16 else f32
    i32 = mybir.dt.int32
    indirect = gather_mode == "indirect"
    if indirect:
        idx_cols = nt * eb  # [128, nt*eb] int32, one idx per (partition, slot)
    nc = bacc.Bacc("TRN2")
    xt_ext = nc.declare_dram_parameter("xt", [128, n_pad], xdt, isOutput=False)
    wt_ext = nc.declare_dram_parameter("wt", [128, 128], xdt, isOutput=False)
    idt = i32 if indirect else i16
    si_ext = nc.declare_dram_parameter("si", [128, idx_cols], idt, isOutput=False)
    di_ext = nc.declare_dram_parameter("di", [128, idx_cols], idt, isOutput=False)
    out_ext = nc.declare_dram_parameter("out", [128, nt * eb], f32, isOutput=True)
    q_hbm = nc.dram_tensor("qbuf", [nq, 128, 128], f32)

    q_rows = q_hbm[:].rearrange("t p c -> (t p) c")  # [n_pad, 128] node-major
    # strided pair views of Q rows: row-pair p covers nodes 2p / 2p+1
    q_pairs = q_hbm[:].rearrange("t p c -> (t p c)").rearrange("(n x) -> n x", x=256)
    if elem_words == 128:
        q_views = [q_pairs[:, 0:128], q_pairs[:, 128:256]]  # even / odd nodes
    else:  # timing-only mode: gather whole pairs
        q_views = [q_pairs[:, 0:elem_words], q_pairs[:, 0:elem_words]]

    from contextlib import ExitStack

    with ExitStack() as ctx:
        wt_sb = ctx.enter_context(nc.sbuf_tensor("wt_sb", [128, 128], xdt))
        si_sb = ctx.enter_context(nc.sbuf_tensor("si_sb", [128, idx_cols], idt))
        di_sb = ctx.enter_context(nc.sbuf_tensor("di_sb", [128, idx_cols], idt))
        xtb = ctx.enter_context(nc.sbuf_tensor("xtb", [128, 2, 128], xdt))
        q_sb = ctx.enter_context(nc.sbuf_tensor("q_sb", [128, 2, 128], f32))
        S = ctx.enter_context(nc.sbuf_tensor("S", [128, 2, eb, elem_words], f32))
        D = ctx.enter_context(nc.sbuf_tensor("D", [128, 2, eb, elem_words], f32))
        P = ctx.enter_context(nc.sbuf_tensor("P", [128, 3, eb, 128], f32))
        out_sc = ctx.enter_context(nc.sbuf_tensor("out_sc", [128, nt * eb], f32))
        qp0 = ctx.enter_context(nc.psum_tensor("qp0", [128, 512], f32))
        qp1 = ctx.enter_context(nc.psum_tensor("qp1", [128, 512], f32))
        ld = ctx.enter_context(nc.semaphore("ld"))  # si/di/wt loads
        xl0 = ctx.enter_context(nc.semaphore("xl0"))  # xt loads even
        xl1 = ctx.enter_context(nc.semaphore("xl1"))
        pe = ctx.enter_context(nc.semaphore("pe"))  # matmuls
        cpA = ctx.enter_context(nc.semaphore("cpA"))  # ACT copies (even)
        cpD = ctx.enter_context(nc.semaphore("cpD"))  # DVE copies (odd)
        qs0 = ctx.enter_context(nc.semaphore("qs0"))  # q stores even
        qs1 = ctx.enter_context(nc.semaphore("qs1"))
        gt0 = ctx.enter_context(nc.semaphore("gt0"))  # gathers even tiles
        gt1 = ctx.enter_context(nc.semaphore("gt1"))
        ed = ctx.enter_context(nc.semaphore("ed"))  # DVE edge ops
        os_ = ctx.enter_context(nc.semaphore("os"))
        block = ctx.enter_context(nc.Block())
        qp = [qp0, qp1]
        xl = [xl0, xl1]
        qs = [qs0, qs1]
        gt = [gt0, gt1]

        def wait_copy(eng, u, thresh_tiles):
            """wait until psum->sbuf copy of Q tile `u` is done"""
            if u % 2 == 0:
                eng.wait_ge(cpA, evens(u + 1))
            else:
                eng.wait_ge(cpD, odds(u + 1))

        @block.sync
        def _(sync):
            sync.dma_start(out=wt_sb[:], in_=wt_ext[:]).then_inc(ld, 16)
            sync.dma_start(out=si_sb[:], in_=si_ext[:]).then_inc(ld, 16)
            sync.dma_start(out=di_sb[:], in_=di_ext[:]).then_inc(ld, 16)
            for u in range(NQT):
                r, k = divmod(u, nq)
                if u >= 2:
                    sync.wait_ge(pe, u - 1)  # matmul u-2 done -> xtb[u%2] free
                sync.dma_start(
                    out=xtb[:, u % 2, :], in_=xt_ext[:, k * 128 : (k + 1) * 128]
                ).then_inc(xl[u % 2], 16)
                if u >= 1:
                    j = u - 1
                    if do_edges and j % nq == 0 and j >= nq:
                        # this store is rep r's first q_hbm write: rep r-1
                        # gathers (which read q_hbm) must be done first
                        rr = j // nq
                        sync.wait_ge(gt[0], 16 * nt * rr)
                        sync.wait_ge(gt[1], 16 * nt * rr)
                    wait_copy(sync, j, None)
                    sync.dma_start(
                        out=q_hbm[(u - 1) % nq], in_=q_sb[:, j % 2, :]
                    ).then_inc(qs[j % 2], 16)
            if NQT:
                j = NQT - 1
                wait_copy(sync, j, None)
                sync.dma_start(out=q_hbm[j % nq], in_=q_sb[:, j % 2, :]).then_inc(
                    qs[j % 2], 16
                )
            # final scores out
            if NTT:
                if do_dve:
                    sync.wait_ge(ed, 2 * NTT)
                else:
                    n_even = (NTT + 1) // 2
                    sync.wait_ge(gt[0], 16 * n_gath * n_even)
                    sync.wait_ge(gt[1], 16 * n_gath * (NTT - n_even))
            sync.dma_start(out=out_ext[:], in_=out_sc[:]).then_inc(os_, 16)
            sync.wait_ge(os_, 16)

        @block.tensor
        def _(tensor):
            if NQT:
                tensor.wait_ge(ld, 48)  # wt loaded (also si/di; single sem)
            for u in range(NQT):
                tensor.wait_ge(xl[u % 2], 16 * (u // 2 + 1))
                if u >= 2:
                    wait_copy(tensor, u - 2, None)  # psum bank evacuated
                tensor.matmul(
                    qp[u % 2][:, 0:128], lhsT=xtb[:, u % 2, :], rhs=wt_sb[:]
                ).then_inc(pe, 1)

        @block.scalar
        def _(scalar):
            for u in range(0, NQT, 2):
                scalar.wait_ge(pe, u + 1)
                if u >= 2:
                    scalar.wait_ge(qs[0], 16 * (u // 2))  # q_sb[0] stored (u-2)
                scalar.copy(out=q_sb[:, 0, :], in_=qp[0][:, 0:128]).then_inc(cpA, 1)

        def mult_count(u):
            # ed value after mult(u) completes (per-rep flushed pipeline)
            r, t = divmod(u, nt)
            return 2 * r * nt + (2 * t if t >= 1 else 1)

        def red_count(u):
            # ed value after red(u) completes
            r, t = divmod(u, nt)
            return 2 * r * nt + (2 * t + 3 if t < nt - 1 else 2 * nt)

        @block.vector
        def _(vector):
            import concourse.mybir as mybir

            def red(u):
                # DVE is pipelined: a same-engine RAW on P needs an explicit
                # (instantly-satisfied) wait on the producing mult
                vector.wait_ge(ed, mult_count(u))
                t = u % nt
                vector.tensor_reduce(
                    out=out_sc[:, t * eb : (t + 1) * eb],
                    in_=P[:, u % 3, :, :],
                    axis=mybir.AxisListType.X,
                    op=mybir.AluOpType.add,
                ).then_inc(ed, 1)

            for r in range(max(reps, 1)):
                # rep r: Q-phase PSUM->SBUF copies of odd tiles
                if do_q:
                    for u in range(r * nq, (r + 1) * nq):
                        if u % 2 == 0:
                            continue
                        vector.wait_ge(pe, u + 1)
                        if u >= 2:
                            vector.wait_ge(qs[1], 16 * (u // 2))
                        vector.tensor_copy(
                            out=q_sb[:, 1, :], in_=qp[1][:, 0:128]
                        ).then_inc(cpD, 1)
                # rep r: edge phase (software pipeline: mult(u) then red(u-1)
                # so same-buffer DVE ops are never pipeline-adjacent)
                if do_edges and do_dve:
                    for u in range(r * nt, (r + 1) * nt):
                        t = u % nt
                        vector.wait_ge(gt[u % 2], 16 * n_gath * (u // 2 + 1))
                        if u >= 3:
                            # WAR: P[u%3] was last read by red(u-3)
                            vector.wait_ge(ed, red_count(u - 3))
                        vector.tensor_tensor(
                            out=P[:, u % 3, :, :],
                            in0=S[:, u % 2, :, :],
                            in1=D[:, u % 2, :, :],
                            op=mybir.AluOpType.mult,
                        ).then_inc(ed, 1)
                        if t >= 1:
                            red(u - 1)
                    red((r + 1) * nt - 1)

        @block.gpsimd
        def _(gpsimd):
            if NTT:
                gpsimd.load_library(mlp)
                gpsimd.wait_ge(ld, 48)  # si/di in SBUF
            ic = tile_e // 16  # idx cols per tile
            import math

            def q_tiles_needed(tau):
                # edges in each parity group are host-sorted by max(src,dst):
                # tile tau only touches Q rows below the (tau+1)/tpg quantile
                # of max(u,v) with u,v ~ U[0,N): P(max<=m) = (m/N)^2.
                # +16 tiles of safety margin (~18 sigma of order-stat noise).
                return min(nq, int(nq * math.sqrt((tau + 1) / tiles_per_group)) + 16)

            for u in range(NTT):
                r, t = divmod(u, nt)
                if do_q and overlap_q:
                    need = r * nq + q_tiles_needed(t % tiles_per_group)
                    gpsimd.wait_ge(qs[0], 16 * evens(need))
                    gpsimd.wait_ge(qs[1], 16 * odds(need))
                elif do_q and t == 0:
                    # Q of rep r fully written
                    gpsimd.wait_ge(qs[0], 16 * evens((r + 1) * nq))
                    gpsimd.wait_ge(qs[1], 16 * odds((r + 1) * nq))
                if u >= 2 and do_dve:
                    # mult(u-2) (which read S/D[u%2]) must be done
                    gpsimd.wait_ge(ed, mult_count(u - 2))
                if indirect:
                    import concourse.bass as bass

                    gpsimd.indirect_dma_start(
                        out=S[:, u % 2, :, :],
                        out_offset=None,
                        in_=q_rows,
                        in_offset=bass.IndirectOffsetOnAxis(
                            ap=si_sb[:, t * eb : (t + 1) * eb], axis=0
                        ),
                    ).then_inc(gt[u % 2], 16)
                    if n_gath == 2:
                        gpsimd.indirect_dma_start(
                            out=D[:, u % 2, :, :],
                            out_offset=None,
                            in_=q_rows,
                            in_offset=bass.IndirectOffsetOnAxis(
                                ap=di_sb[:, t * eb : (t + 1) * eb], axis=0
                            ),
                        ).then_inc(gt[u % 2], 16)
                    continue
                g = t // tiles_per_group
                sp, dp = g >> 1, g & 1
                gpsimd.dma_gather(
                    S[:, u % 2, :, :],
                    q_views[sp],
                    si_sb[:, t * ic : (t + 1) * ic],
                    tile_e,
                    tile_e,
                    elem_words,
                    elem_step=256,
                    single_packet=SINGLE_PACKET,
                ).then_inc(gt[u % 2], 16)
                if n_gath == 2:
                    gpsimd.dma_gather(
                        D[:, u % 2, :, :],
                        q_views[dp],
                        di_sb[:, t * ic : (t + 1) * ic],
                        tile_e,
                        tile_e,
                        elem_words,
                        elem_step=256,
                        single_packet=SINGLE_PACKET,
                    ).then_inc(gt[u % 2], 16)

    nc.compile()
    return nc


_GRAPH_CACHE = {}


def _get_graph(nq, tiles_per_group, tile_e, overlap_q=True):
    key = (nq, tiles_per_group, tile_e, overlap_q)
    if key not in _GRAPH_CACHE:
        _GRAPH_CACHE[key] = build_graph(
            nq, tiles_per_group, tile_e, overlap_q=overlap_q
        )
    return _GRAPH_CACHE[key]


def _wrap_idx(arr):
    """[n] int16 -> SWDGE SBUF index layout [128, n//16]."""
    a = arr.reshape(-1, 16).T  # [16, n/16]
    return np.tile(a, (8, 1)).copy()  # replicate for the 8 Q7 cores


def shard_edges(edge_index, tiles_per_group=TILES_PER_GROUP, tile_e=TILE_E,
                sort_for_overlap=False, nq=NQ):
    """Returns per-core (si, di) index planes + per-core edge positions."""
    import math

    cap = tiles_per_group * tile_e
    s = edge_index[0].astype(np.int64)
    d = edge_index[1].astype(np.int64)
    g = ((s & 1) << 1) | (d & 1)
    si_planes, di_planes, positions = (
        [[] for _ in range(N_CORES)],
        [[] for _ in range(N_CORES)],
        [[] for _ in range(N_CORES)],
    )
    for grp in range(4):
        (pos,) = np.nonzero(g == grp)
        chunks = np.array_split(pos, N_CORES)
        for c, ch in enumerate(chunks):
            n = len(ch)
            assert n <= cap, f"group {grp} core {c}: {n} > cap {cap}"
            if sort_for_overlap:
                ch = ch[np.argsort(np.maximum(s[ch], d[ch]), kind="stable")]
            sh = np.zeros(cap, np.int16)
            dh = np.zeros(cap, np.int16)
            sh[:n] = (s[ch] >> 1).astype(np.int16)
            dh[:n] = (d[ch] >> 1).astype(np.int16)
            pp = np.full(cap, -1, np.int64)
            pp[:n] = ch
            if sort_for_overlap:
                # verify the builder's per-tile Q-prefix assumption holds
                mx = np.maximum(s[ch], d[ch])
                for tau in range(tiles_per_group):
                    lo, hi = tau * tile_e, min((tau + 1) * tile_e, n)
                    if lo >= n:
                        break
                    need = min(nq, int(nq * math.sqrt((tau + 1) / tiles_per_group)) + 16)
                    tile_max = int(mx[lo:hi].max())
                    assert tile_max < need * 128, (
                        f"overlap sort violated: grp {grp} core {c} tile {tau}: "
                        f"max node {tile_max} >= {need * 128}"
                    )
            si_planes[c].append(sh)
            di_planes[c].append(dh)
            positions[c].append(pp)
    out = []
    for c in range(N_CORES):
        out.append(
            (
                _wrap_idx(np.concatenate(si_planes[c])),
                _wrap_idx(np.concatenate(di_planes[c])),
                np.concatenate(positions[c]),
            )
        )
    return out


def shard_edges_indirect(edge_index, tiles_per_group=TILES_PER_GROUP, tile_e=TILE_E):
    """int32 index planes for gather_mode=indirect: no parity groups.
    Returns per-core (si, di, n_valid); edges dealt contiguously."""
    nt = 4 * tiles_per_group
    eb = tile_e // 128
    cap = nt * tile_e
    s = edge_index[0].astype(np.int64)
    d = edge_index[1].astype(np.int64)
    n = s.shape[0]
    out = []
    bounds = np.linspace(0, n, N_CORES + 1).astype(np.int64)
    for c in range(N_CORES):
        lo, hi = bounds[c], bounds[c + 1]
        m = hi - lo
        assert m <= cap
        sp = np.zeros(cap, np.int32)
        dp = np.zeros(cap, np.int32)
        sp[:m] = s[lo:hi]
        dp[:m] = d[lo:hi]

        def plane(arr):
            a = arr.reshape(nt, 128, eb)  # [t, p, j]
            return np.ascontiguousarray(a.transpose(1, 0, 2).reshape(128, nt * eb))

        out.append((plane(sp), plane(dp), int(m)))
    return out


def kernel(x, edge_index, W):
    from concourse.bass_utils import run_bass_kernel_spmd

    x = np.asarray(x)
    edge_index = np.asarray(edge_index)
    W = np.asarray(W)

    xt = np.zeros((N_FEAT, N_PAD), np.float32)
    xt[:, :N_NODES] = np.ascontiguousarray(x.astype(np.float32).T)
    wt = np.ascontiguousarray(W.astype(np.float32).T)

    shards = shard_edges(edge_index, sort_for_overlap=True)
    in_maps = [
        {"xt": xt, "wt": wt, "si": si, "di": di} for (si, di, _pos) in shards
    ]

    nc = _get_graph(NQ, TILES_PER_GROUP, TILE_E, overlap_q=True)
    res = run_bass_kernel_spmd(nc, in_maps, core_ids=list(range(N_CORES)))
    global LAST_RESULT
    LAST_RESULT = res

    nt = 4 * TILES_PER_GROUP
    scores = np.empty(N_EDGES, np.float32)
    for c in range(N_CORES):
        buf = res.results[c]["out"]  # [128, nt*EB]
        flat = buf.reshape(128, nt, EB).transpose(1, 2, 0).reshape(-1)
        pos = shards[c][2]
        m = pos >= 0
        scores[pos[m]] = flat[m]
    return scores
